# revision 8
# baseline (speedup 1.0000x reference)
"""Trainium2 Bass kernel for nn_AttentionModule (retrieval_knn).

reference math:
    S = support.reshape(B, N*K, D)
    dist_sq[b,q,nk] = -sum_d (S[b,nk,d] - query[b,q,d])^2
    qgw = softmax_K(tanh(mean_q dist_sq))          # (B,N,K,1)
    agg = sum_k support * qgw                      # (B,N,D)

The mean over q commutes with the squared-distance expansion:
    mean_q dist_sq[b,nk] = 2*S.qbar - ||S||^2 - mean_q ||q||^2
with qbar = mean_q query[b].  So the kernel only needs one streaming
pass over query (the memory-bound part) plus a tiny support-side tail.

Sharding: pure data parallel over B=4 episodes (cores 4-7 duplicate).
"""

import numpy as np

import concourse.bass as bass
import concourse.tile as tile
from concourse import mybir
from concourse.bass_utils import run_bass_kernel_spmd

B, NN, KK, Q, D = 4, 10, 5, 512, 1536
NK = NN * KK  # 50
QT = Q // 128  # 4 query tiles of 128 rows
F32 = mybir.dt.float32
AF = mybir.ActivationFunctionType
ALU = mybir.AluOpType


def _split_multiwait(nc: bass.Bass) -> None:
    """This container's walrus only supports one sem-wait per instruction;
    Tile's final drain carries several. Hoist extra waits onto dedicated
    single-wait event-semaphore instructions right before the offender."""
    for f in nc.m.functions:
        for b in f.blocks:
            new_insts = []
            for ins in b.instructions:
                si = ins.sync_info
                if si is not None and len(si.on_wait) > 1:
                    waits = list(si.on_wait)
                    for i, w in enumerate(waits[:-1]):
                        ev = mybir.InstEventSemaphore(
                            name=f"{ins.name}-mw{i}",
                            engine=ins.engine,
                            sync_info=mybir.SyncInfo(on_wait=[w], on_update=[]),
                        )
                        new_insts.append(ev)
                    si.on_wait = waits[-1:]
                new_insts.append(ins)
            b.instructions[:] = new_insts


def _build_program(split_multiwait: bool = True) -> bass.Bass:
    nc = bass.Bass()

    q_in = nc.declare_dram_parameter("q", [Q, D], F32, isOutput=False)
    s_in = nc.declare_dram_parameter("s", [NK, D], F32, isOutput=False)
    g_in = nc.declare_dram_parameter("g", [NK, NN], F32, isOutput=False)
    gt_in = nc.declare_dram_parameter("gt", [NN, NK], F32, isOutput=False)
    agg_out = nc.declare_dram_parameter("agg", [NN, D], F32, isOutput=True)
    qgw_out = nc.declare_dram_parameter("qgw", [NK, 1], F32, isOutput=True)

    with tile.TileContext(nc) as tc:
        with (
            tc.tile_pool(name="consts", bufs=1) as consts,
            tc.tile_pool(name="qpool", bufs=3) as qpool,
            tc.tile_pool(name="scr", bufs=2) as scr,
            tc.tile_pool(name="stats", bufs=1) as stats,
            tc.tile_pool(name="psum_qb", bufs=2, space="PSUM") as psum_qb_pool,
            tc.tile_pool(name="psum_small", bufs=2, space="PSUM") as psum_small,
            tc.tile_pool(name="outp", bufs=1) as outp,
        ):
            # --- constants / support-side prologue (overlaps query DMA) ---
            ones_b = consts.tile([128, NK], F32)  # lhsT: broadcast+scale 2/Q
            nc.vector.memset(ones_b, 2.0 / Q)
            inv_qb = consts.tile([128, NK], F32)  # lhsT: broadcast+scale 1/Q
            nc.vector.memset(inv_qb, 1.0 / Q)

            s_sb = consts.tile([NK, D], F32)
            nc.sync.dma_start(out=s_sb, in_=s_in[:, :])
            g_sb = consts.tile([NK, NN], F32)
            nc.sync.dma_start(out=g_sb, in_=g_in[:, :])
            gt_sb = consts.tile([NN, NK], F32)
            nc.sync.dma_start(out=gt_sb, in_=gt_in[:, :])

            # ||S||^2 per support vector: (NK,1) via ACT square+accumulate
            sq_s = stats.tile([NK, 1], F32)
            junk0 = scr.tile([NK, D], F32, name="junk0")
            nc.scalar.activation(
                out=junk0, in_=s_sb, func=AF.Square, accum_out=sq_s
            )

            rowsq = stats.tile([128, QT], F32)  # per-query-row ||q||^2
            # qsum broadcast to NK partitions, scaled by 2/Q, accumulated
            # over all query tiles in PSUM
            pqb = psum_qb_pool.tile([NK, D], F32, name="pqb", tag="pqb")

            # --- streaming pass over query ---
            for t in range(QT):
                qt = qpool.tile([128, D], F32, name="qt")
                nc.sync.dma_start(out=qt, in_=q_in[t * 128 : (t + 1) * 128, :])

                # row sums of squares on ACT (frees DVE)
                sq_scr = scr.tile([128, D], F32, name="sq_scr")
                nc.scalar.activation(
                    out=sq_scr,
                    in_=qt,
                    func=AF.Square,
                    accum_out=rowsq[:, t : t + 1],
                )

                # qsum_t broadcast to NK partitions, scaled by 2/Q (PE),
                # accumulated in PSUM over tiles
                for c in range(D // 512):
                    nc.tensor.matmul(
                        pqb[:, c * 512 : (c + 1) * 512],
                        lhsT=ones_b,
                        rhs=qt[:, c * 512 : (c + 1) * 512],
                        start=(t == 0),
                        stop=(t == QT - 1),
                    )

            # --- tail ---
            # mean ||q||^2 broadcast to NK partitions (scaled 1/Q)
            pssq = psum_small.tile([NK, QT], F32, tag="small")
            nc.tensor.matmul(pssq, lhsT=inv_qb, rhs=rowsq, start=True, stop=True)
            ssq_r = stats.tile([NK, 1], F32)
            nc.vector.tensor_reduce(
                out=ssq_r, in_=pssq, axis=mybir.AxisListType.X, op=ALU.add
            )

            # dot[nk] = (2/Q) * sum_d S[nk,d] * qsum[d]
            junk = scr.tile([NK, D], F32, name="junk")
            nc.vector.tensor_mul(junk, s_sb, pqb)
            dotq = stats.tile([NK, 1], F32)
            nc.vector.tensor_reduce(
                out=dotq, in_=junk, axis=mybir.AxisListType.X, op=ALU.add
            )

            # md = 2*S.qbar - ||S||^2 - mean||q||^2
            md = stats.tile([NK, 1], F32)
            nc.vector.tensor_sub(md, dotq, sq_s)
            nc.vector.tensor_sub(md, md, ssq_r)

            th = stats.tile([NK, 1], F32)
            nc.scalar.activation(out=th, in_=md, func=AF.Tanh)
            ex = stats.tile([NK, 1], F32)
            nc.scalar.activation(out=ex, in_=th, func=AF.Exp)

            # group-of-K softmax denominator via block-one-hot matmuls
            pden = psum_small.tile([NN, 1], F32, tag="small")
            nc.tensor.matmul(pden, lhsT=g_sb, rhs=ex, start=True, stop=True)
            den_sb = stats.tile([NN, 1], F32)
            nc.vector.tensor_copy(den_sb, pden)
            pdenb = psum_small.tile([NK, 1], F32, tag="small")
            nc.tensor.matmul(pdenb, lhsT=gt_sb, rhs=den_sb, start=True, stop=True)
            rden = stats.tile([NK, 1], F32)
            nc.vector.reciprocal(rden, pdenb)

            w = stats.tile([NK, 1], F32)
            nc.vector.tensor_mul(w, ex, rden)
            nc.sync.dma_start(out=qgw_out[:, :], in_=w)

            # weighted aggregation: agg[n,d] = sum_k w[n,k] * S[n,k,d]
            wg = stats.tile([NK, NN], F32)
            nc.vector.tensor_scalar(
                out=wg, in0=g_sb, scalar1=w, scalar2=None, op0=ALU.mult
            )
            pagg = psum_qb_pool.tile([NN, D], F32, tag="pqb")
            for c in range(D // 512):
                nc.tensor.matmul(
                    pagg[:, c * 512 : (c + 1) * 512],
                    lhsT=wg,
                    rhs=s_sb[:, c * 512 : (c + 1) * 512],
                    start=True,
                    stop=True,
                )
            agg_sb = outp.tile([NN, D], F32)
            nc.vector.tensor_copy(agg_sb, pagg)
            nc.sync.dma_start(out=agg_out[:, :], in_=agg_sb)

    if split_multiwait:
        _split_multiwait(nc)
    return nc


_NC_CACHE: bass.Bass | None = None


def _get_nc() -> bass.Bass:
    global _NC_CACHE
    if _NC_CACHE is None:
        _NC_CACHE = _build_program()
    return _NC_CACHE


def _host_inputs(support: np.ndarray, query: np.ndarray):
    g = np.zeros((NK, NN), dtype=np.float32)
    for n in range(NN):
        g[n * KK : (n + 1) * KK, n] = 1.0
    gt = np.ascontiguousarray(g.T)
    in_maps = []
    for core in range(8):
        b = core % B
        in_maps.append(
            {
                "q": np.ascontiguousarray(query[b], dtype=np.float32),
                "s": np.ascontiguousarray(
                    support[b].reshape(NK, D), dtype=np.float32
                ),
                "g": g,
                "gt": gt,
            }
        )
    return in_maps


def kernel(support, query, N=NN, K=KK, total_Q=Q, _trace=False):
    support = np.asarray(support, dtype=np.float32)
    query = np.asarray(query, dtype=np.float32)
    assert support.shape == (B, NN, KK, D)
    assert query.shape == (B, Q, D)

    nc = _get_nc()
    in_maps = _host_inputs(support, query)
    res = run_bass_kernel_spmd(nc, in_maps, list(range(8)), trace=_trace)

    agg = np.stack([res.results[b]["agg"] for b in range(B)])
    qgw = np.stack([res.results[b]["qgw"] for b in range(B)])
    out_agg = agg.astype(np.float32)
    out_qgw = qgw.reshape(B, NN, KK, 1).astype(np.float32)
    if _trace:
        return (out_agg, out_qgw), res
    return out_agg, out_qgw
